# revision 17
# baseline (speedup 1.0000x reference)
"""Contrastive-loss kernel for trn2 (8 NeuronCores, SPMD).

The reference loss reduces to a Gram matrix G = F.T @ F over the
flattened input F [N=524288, T=64] (128 MiB fp32), followed by a tiny
[64,64] masked margin reduction.  Row order is irrelevant (the Gram
sums symmetrically over rows), so each core's shard is laid out
host-side as a [128, 32768] fp8-e4m3 image (partition p holds 512
consecutive rows), streamed to SBUF, and reduced on the PE with
double-row fp8 matmuls (K=256 per instruction, 2x column throughput)
into one fp32 PSUM accumulator.  The 8 partial [64,64] Grams are
summed on the host, where the masked margin reduction (negligible
work) also runs.

Precision: e4m3 quantization of the N(0,1) input gives a loss
relative error of ~7e-4 (measured against the fp64 reference on the
harness seed) — 28x inside the 2e-2 gate.  The dominant term is the
deterministic ~0.1% inflation of E[x^2] under 3-bit-mantissa
round-to-nearest; margin hinges stay identically zero because the
pairwise distances (~1e6) dwarf the 6e4 margin.

Each double-row matmul takes lhsT = rhs = [128p, 2k, 128c]: two
128-row k-subtiles whose 128 columns are two 64-col chunks [A|B]
(subtile 0) and [C|D] (subtile 1).  It accumulates A'A+C'C and
B'B+D'D into the two diagonal 64x64 PSUM blocks (off-diagonal
products are discarded), i.e. 512 input rows per instruction.

Structure notes:
  - Straight-line code, no nc.Block(): the Block-exit all-engine
    barrier would serialize the NRT end-of-NEFF semaphore-check storm
    (~7us) after the kernel body.
  - All tiles have dedicated SBUF slots (32 KiB/partition total): no
    recycling, no backpressure sems; every DMA issues up front on the
    gpsimd SWDGE queue (the SP/ACT HWDGE queues only sustain
    ~100 GB/s; SWDGE does ~400 GB/s).  Head tiles are graded small so
    PE ramps early.
  - Per-tile completion sems (a DMA's 16 descriptor-completions can
    interleave with the next DMA's on the same queue), allocated
    contiguously so teardown is one reset-drain + one range clear.
  - Teardown (wait for the output store, reset DMA sem state, clear
    kernel sems so the NEFF can re-execute) runs on the SP engine,
    which has much lower semaphore-wake latency than gpsimd.
"""

import numpy as np
import ml_dtypes

import concourse.bacc as bacc
import concourse.mybir as mybir
from concourse.bass_utils import run_bass_kernel_spmd

MARGIN = 60000.0
S = 64                      # time steps (Gram dim)
N_TOTAL = 2 * 8 * 32 * 32 * 32   # 524288 flattened rows
N_CORES = 8
N_SHARD = N_TOTAL // N_CORES     # 65536 rows per core
P = 128                     # SBUF partitions
TOTAL_FREE = N_SHARD * S // P    # 32768 fp8 elems per partition
# Per-tile free-dim sizes (elems/partition, multiples of 256 = one
# double-row matmul).  Head graded small so PE ramps early; tail
# graded small so PE's last matmuls trail the stream closely; 8 KiB
# body lines amortize per-packet DMA overhead.
TILE_FREE = [1024, 3072, 8192, 8192, 4096, 4096, 2048, 2048]
assert sum(TILE_FREE) == TOTAL_FREE
N_TILES = len(TILE_FREE)
TILE_OFF = np.cumsum([0] + TILE_FREE).tolist()

_CACHE = {}
LAST_RESULTS = None         # BassKernelResults of the most recent run


def _build_nc():
    nc = bacc.Bacc("TRN2", target_bir_lowering=False, debug=False,
                   num_devices=N_CORES)
    x = nc.dram_tensor("x", [P, TOTAL_FREE], mybir.dt.float8e4,
                       kind="ExternalInput")
    g = nc.dram_tensor("g", [S, S], mybir.dt.float32, kind="ExternalOutput")

    with (
        nc.sbuf_tensor("xbuf", [P, TOTAL_FREE], mybir.dt.float8e4) as xbuf,
        nc.psum_tensor("acc", [2 * S, 2 * S], mybir.dt.float32) as acc,
        nc.sbuf_tensor("obuf", [S, S], mybir.dt.float32) as obuf,
    ):
        import contextlib
        with contextlib.ExitStack() as stack:
            tsems = [stack.enter_context(nc.semaphore(f"ts{k}"))
                     for k in range(N_TILES)]
            pe_sem = stack.enter_context(nc.semaphore("pe_sem"))
            out_sem = stack.enter_context(nc.semaphore("out_sem"))
            fin_sem = stack.enter_context(nc.semaphore("fin_sem"))

            nums = [s.num for s in tsems + [pe_sem, out_sem, fin_sem]]
            assert nums == list(range(nums[0], nums[0] + len(nums))), nums
            sem_range = range(nums[0], nums[-1] + 1)

            # --- input DMAs, issued up front.  Tile 0 rides the SP HWDGE
            # queue (slow, ~100 GB/s, but it starts ~0.8us before gpsimd
            # and the tile is tiny, so PE ramps earliest); the rest go on
            # gpsimd SWDGE (~400 GB/s).
            for k in range(N_TILES):
                eng = nc.sync if k == 0 else nc.gpsimd
                eng.dma_start(
                    xbuf[:, TILE_OFF[k]:TILE_OFF[k + 1]],
                    x[:, TILE_OFF[k]:TILE_OFF[k + 1]],
                ).then_inc(tsems[k], 16)

            # --- PE: Gram accumulation (double-row fp8 matmuls) -----------
            for k in range(N_TILES):
                nc.tensor.wait_ge(tsems[k], 16)
                n_dr = TILE_FREE[k] // 256
                for j in range(n_dr):
                    c = xbuf[:, TILE_OFF[k] + j * 256:
                             TILE_OFF[k] + (j + 1) * 256].rearrange(
                                 "p (k c) -> p k c", k=2)
                    mm = nc.tensor.matmul(
                        acc[:], c, c,
                        start=(k == 0 and j == 0),
                        stop=(k == N_TILES - 1 and j == n_dr - 1),
                        perf_mode=mybir.MatmulPerfMode.DoubleRow,
                    )
                    if k == N_TILES - 1 and j == n_dr - 1:
                        mm.then_inc(pe_sem, 1)

            # --- DVE: reduce the PSUM diag blocks into obuf ---------------
            # (two steps: TensorTensor may read only one input from PSUM)
            nc.vector.wait_ge(pe_sem, 1)
            nc.vector.tensor_copy(obuf[:], acc[:S, :S])
            nc.vector.tensor_add(obuf[:], obuf[:],
                                 acc[S:, S:]).then_inc(out_sem, 1)

            # --- SP: store the partial Gram, then teardown ----------------
            # (walrus requires a sync update on every dynamic DMA, and the
            # completion sem must be waited out before the range clear so
            # the NEFF's end-of-execution sem-zero checks don't hang)
            nc.sync.wait_ge(out_sem, 1)
            nc.sync.dma_start(g[:], obuf[:]).then_inc(fin_sem, 16)
            nc.sync.wait_ge(fin_sem, 16)
            nc.sync.drain(semaphore_range=nc._kernel_sem_range)
            nc.sync.sem_clear(sem_range)

    nc.compile()
    return nc


def get_nc():
    if "nc" not in _CACHE:
        _CACHE["nc"] = _build_nc()
    return _CACHE["nc"]


def _device_partial_grams(shards: np.ndarray, **run_kwargs) -> np.ndarray:
    """Run the SPMD bass kernel; return the 8 partial Grams [8, 64, 64]."""
    global LAST_RESULTS
    nc = get_nc()
    in_maps = [{"x": shards[c]} for c in range(N_CORES)]
    LAST_RESULTS = run_bass_kernel_spmd(
        nc, in_maps, core_ids=list(range(N_CORES)), **run_kwargs
    )
    return np.stack([LAST_RESULTS.results[c]["g"] for c in range(N_CORES)])


def kernel(input: np.ndarray, **run_kwargs) -> np.ndarray:
    # Shard prep: core c takes rows [c*65536, (c+1)*65536) of the
    # flattened [N, 64] input; partition p of core c holds 512
    # consecutive rows as one contiguous fp8 line.
    shards = np.ascontiguousarray(
        np.asarray(input).reshape(N_CORES, P, TOTAL_FREE)
    ).astype(ml_dtypes.float8_e4m3)
    partials = _device_partial_grams(shards, **run_kwargs)

    gram = partials.astype(np.float64).sum(axis=0)
    sq = np.diag(gram)
    dist = sq[:, None] + sq[None, :] - 2.0 * gram
    idx = np.arange(S)
    lower = idx[:, None] > idx[None, :]
    adjacent = (idx[:, None] - idx[None, :]) == 1
    per_pair = np.where(adjacent, np.maximum(0.0, MARGIN - dist), dist)
    loss = np.where(lower, per_pair, 0.0).sum() / (S * (S - 1) * 1000)
    return np.asarray(loss, dtype=np.float32)
